# revision 32
# baseline (speedup 1.0000x reference)
"""Multi-head attention (RoPE, dense mask) Trainium2 Bass kernel.

Problem: B=2, S=2048, D=1024, H=16 heads of depth 64.
  q/k/v = query @ W{q,k,v}.T + b   (RoPE on q,k)   -> softmax(q k^T / 8) v
  out = gamma * (attn @ Wo.T + bo)

Sharding over 8 cores: batch (2) x head-groups (4 heads = 256 dims each).
Each core computes its batch's attention for its 4 heads plus the partial
row-parallel out-projection; host sums the 4 partials per batch.

The kernel is softmax-exp bound: 16.8M exps/core on the scalar (ACT)
engine at 1 elem/lane/cycle (1.2 GHz) is ~135us. The design keeps ACT
busy on exp and hides everything else under it:
  - all matmuls bf16 (fp8 anywhere in the q/k/wt/v path costs ~2.5e-2
    rel err: attention output is a weighted average, so quantization
    noise does NOT average down relative to the signal).
  - logits (contraction=64) issue head-pairs back-to-back into disjoint
    PE row-groups (partitions 0-63 / 64-127): the two matmuls run
    concurrently in the array (measured dt_start ~4ns).
  - projections are pipelined INTO the attention passes: pass (qc,mt)
    consumes K-chunks incrementally per k-tile-pair, so K(mt1)/Q/V
    projections are emitted as 'extra' work inside earlier passes.
    Evacuations/bias-adds run on DVE/GPSIMD, never ACT.
  - AV matmuls are emitted 2 k-tile-pairs late so the pass-boundary
    at-psum reuse (next AV start waits previous normalize) hides
    behind ~4us of queued exps.
  - all dram inputs are host-swizzled partition-major so every DMA has
    >=2KB contiguous per-partition lines (~3x DMA throughput).
  - out-projection (bf16 partials) + bf16 output DMA per q-chunk.
"""

import numpy as np
import ml_dtypes

import concourse.bass as bass
import concourse.tile as tile
from concourse import bacc, mybir
from concourse.bass_utils import run_bass_kernel_spmd

B, S, D, H, DEPTH = 2, 2048, 1024, 16, 64
N_CORES = 8
HPC = 4            # heads per core
HD = HPC * DEPTH   # 256 head-dims per core
P = 128
KT = D // P        # 8 contraction tiles for the projections
NCH = S // 512     # 4 chunks of 512
TT = S // P        # 16 token/key tiles
KTP = TT // 2      # 8 k-tile pairs per pass
F32 = mybir.dt.float32
BF16 = mybir.dt.bfloat16
EXP = mybir.ActivationFunctionType.Exp
BF16_NP = ml_dtypes.bfloat16

_BUILT = None


def _mha_tile(tc, io):
    nc = tc.nc
    qt, wq, wk, wv, wo = io["qt"], io["wq"], io["wk"], io["wv"], io["wo"]
    bq, bk, cost, sint = io["bq"], io["bk"], io["cost"], io["sint"]
    rotm, bout, out_t = io["rotm"], io["bout"], io["out_t"]

    with tc.tile_pool(name="persist", bufs=1) as persist:
        qTr = [persist.tile([P, S], BF16, tag=f"qTr{m}", name=f"qTr{m}") for m in range(2)]
        kTr = [persist.tile([P, S], BF16, tag=f"kTr{m}", name=f"kTr{m}") for m in range(2)]
        # token-major V with an all-ones 65th column per head (denominator)
        v_sb = persist.tile([P, TT, HPC, DEPTH + 1], BF16, tag="v")
        nc.vector.memset(v_sb[:, :, :, DEPTH : DEPTH + 1], 1.0)
        attn_sb = [persist.tile([P, S], BF16, tag=f"attn{m}", name=f"attn{m}") for m in range(2)]

        # ---- input DMA, ordered for earliest K(mt0) start ----
        rotm_sb = persist.tile([P, P], BF16, tag="rotm")
        nc.sync.dma_start(out=rotm_sb, in_=rotm)
        w_sbs = {}
        for name, w, b in (("wk", wk, bk), ("wq", wq, bq), ("wv", wv, None)):
            w_sbs[name] = persist.tile([P, KT, HD], BF16, tag=name, name=name)
            if b is not None:
                w_sbs[name + "_b"] = persist.tile(
                    [P, 2], F32, tag=name + "_b", name=name + "_b"
                )
        cos_sb = persist.tile([P, 2, S], BF16, tag="cos")
        sin_sb = persist.tile([P, 2, S], BF16, tag="sin")
        qt_sb = persist.tile([P, KT, S], BF16, tag="qt")
        wo_sb = persist.tile([P, 2, D], BF16, tag="wo")
        bout_sb = persist.tile([P, KT], F32, tag="bout")

        nc.sync.dma_start(out=w_sbs["wk"], in_=wk)
        nc.sync.dma_start(out=w_sbs["wk_b"], in_=bk)
        for kt in range(KT):
            nc.sync.dma_start(out=qt_sb[:, kt, 0:1024], in_=qt[:, kt, 0:1024])
        nc.sync.dma_start(out=cos_sb[:, 0], in_=cost[:, 0])
        nc.sync.dma_start(out=sin_sb[:, 0], in_=sint[:, 0])
        nc.sync.dma_start(out=w_sbs["wq"], in_=wq)
        nc.sync.dma_start(out=w_sbs["wq_b"], in_=bq)
        nc.sync.dma_start(out=w_sbs["wv"], in_=wv)
        for kt in range(KT):
            nc.sync.dma_start(out=qt_sb[:, kt, 1024:2048], in_=qt[:, kt, 1024:2048])
        nc.sync.dma_start(out=cos_sb[:, 1], in_=cost[:, 1])
        nc.sync.dma_start(out=sin_sb[:, 1], in_=sint[:, 1])
        nc.sync.dma_start(out=wo_sb, in_=wo)
        nc.sync.dma_start(out=bout_sb, in_=bout)

        with (
            tc.tile_pool(name="wt", bufs=8) as wtp,
            tc.tile_pool(name="bc", bufs=3) as bcp,
            tc.tile_pool(name="rcp", bufs=3) as rcpp,
            tc.tile_pool(name="oc", bufs=3) as ocp,
            tc.tile_pool(name="lg_ps", bufs=2, space="PSUM") as lgp,
            tc.tile_pool(name="at_ps", bufs=1, space="PSUM") as atp,
            tc.tile_pool(name="ms_ps", bufs=2, space="PSUM") as msp,
        ):
            # PE warm-up: HAM clock-gate ramps to 8/8 while input DMAs land.
            warm = msp.tile([P, 512], F32, tag="ms", name="warm")
            for _ in range(48):
                nc.tensor.matmul(
                    warm[:, 0:P], lhsT=rotm_sb, rhs=rotm_sb,
                    start=True, stop=True, skip_group_check=True,
                )

            def emit_proj(wname, dstpair, mt, ch):
                """Q/K projection + RoPE for one (mt half, 512-token chunk)."""
                w_sb, b_sb = w_sbs[wname], w_sbs[wname + "_b"]
                sl = bass.ts(ch, 512)
                dst = dstpair[mt]
                ps = msp.tile([P, 512], F32, tag="ms")
                for kt in range(KT):
                    nc.tensor.matmul(
                        ps,
                        lhsT=w_sb[:, kt, mt * P : (mt + 1) * P],
                        rhs=qt_sb[:, kt, sl],
                        start=(kt == 0),
                        stop=(kt == KT - 1),
                    )
                # evacuate + bias on DVE (keep ACT free for exp)
                nc.vector.tensor_scalar_add(
                    out=dst[:, sl], in0=ps, scalar1=b_sb[:, mt : mt + 1]
                )
                rps = msp.tile([P, 512], F32, tag="ms")
                nc.tensor.matmul(
                    rps, lhsT=rotm_sb, rhs=dst[:, sl], start=True, stop=True
                )
                # x' = x*cos + rot(x)*sin_signed  (all DVE: gpsimd must run a
                # single ucode -- partition_broadcast -- or each program
                # switch costs ~7us of Q7 reload).  cos-mul goes to a temp so
                # it runs during the rot matmul instead of serializing.
                ct = ocp.tile([P, 512], BF16, tag="ct")
                nc.vector.tensor_mul(out=ct, in0=dst[:, sl], in1=cos_sb[:, mt, sl])
                nc.vector.tensor_mul(out=rps, in0=rps, in1=sin_sb[:, mt, sl])
                nc.vector.tensor_add(out=dst[:, sl], in0=ct, in1=rps)

            def emit_v(tt):
                """V projection for one 128-token tile -> bf16 token-major."""
                ps = msp.tile([P, 512], F32, tag="ms")
                for kt in range(KT):
                    nc.tensor.matmul(
                        ps[:, 0:HD],
                        lhsT=qt_sb[:, kt, tt * P : (tt + 1) * P],
                        rhs=w_sbs["wv"][:, kt, :],
                        start=(kt == 0),
                        stop=(kt == KT - 1),
                    )
                nc.vector.tensor_copy(
                    out=v_sb[:, tt, :, 0:DEPTH],
                    in_=ps[:, 0:HD].rearrange("p (h d) -> p h d", h=HPC),
                )

            def emit_outproj(qc, kts=(0, 1), dst_t=None, ots=range(KT)):
                """Out-projection partial over attn mt-halves `kts`.  qc3 is
                split: the mt0 half runs a pass early into out_t2 (host adds
                it), shortening the tail to the mt1 half only."""
                qsl = bass.ts(qc, 512)
                dst_t = out_t if dst_t is None else dst_t
                for ot in ots:
                    ps = msp.tile([P, 512], F32, tag="ms")
                    for kt in kts:
                        nc.tensor.matmul(
                            ps,
                            lhsT=wo_sb[:, kt, ot * P : (ot + 1) * P],
                            rhs=attn_sb[kt][:, qsl],
                            start=(kt == kts[0]),
                            stop=(kt == kts[-1]),
                        )
                    ob = ocp.tile([P, 512], BF16, tag="ob")
                    if kts[-1] == 1:
                        nc.vector.tensor_scalar_add(
                            out=ob, in0=ps, scalar1=bout_sb[:, ot : ot + 1]
                        )
                    else:
                        nc.vector.tensor_copy(out=ob, in_=ps)
                    nc.sync.dma_start(
                        out=dst_t[ot * P : (ot + 1) * P, qc * 512 : (qc + 1) * 512]
                        if dst_t is out_t
                        else dst_t[ot * P : (ot + 1) * P, :],
                        in_=ob,
                    )

            # ---- lead-in: K(mt0, ch0), Q(mt0, ch0) only ----
            # attention passes consume K-chunks and V-tiles incrementally;
            # everything else is emitted as 'extra' work inside the passes.
            emit_proj("wk", kTr, 0, 0)
            emit_proj("wq", qTr, 0, 0)

            # extra work emitted inside attention passes (after each ktp's
            # logits/exps so the ACT queue is fed first): (pass, ktp) -> fns
            H1, H2 = range(0, 4), range(4, KT)
            extra = {
                (0, 0): [lambda: emit_v(0), lambda: emit_v(1)],
                (0, 1): [lambda: emit_proj("wk", kTr, 0, 1),
                         lambda: emit_v(2), lambda: emit_v(3)],
                (0, 2): [lambda: emit_v(4), lambda: emit_v(5)],
                (0, 3): [lambda: emit_proj("wk", kTr, 0, 2)],
                (0, 4): [lambda: emit_v(6), lambda: emit_v(7)],
                (0, 5): [lambda: emit_proj("wk", kTr, 0, 3),
                         lambda: emit_v(8), lambda: emit_v(9)],
                (0, 6): [lambda: emit_v(10), lambda: emit_v(11)],
                (0, 7): [lambda: emit_v(12), lambda: emit_v(13),
                         lambda: emit_proj("wq", qTr, 0, 1)],
                (1, 0): [lambda: emit_v(14), lambda: emit_v(15)],
                (1, 5): [lambda: emit_proj("wk", kTr, 1, 0)],
                (1, 6): [lambda: emit_proj("wk", kTr, 1, 1)],
                (1, 7): [lambda: emit_proj("wq", qTr, 0, 2)],
                (2, 0): [lambda: emit_proj("wk", kTr, 1, 2)],
                (2, 1): [lambda: emit_proj("wk", kTr, 1, 3)],
                (2, 5): [lambda: emit_proj("wq", qTr, 1, 0)],
                (3, 5): [lambda: emit_proj("wq", qTr, 0, 3)],
                (4, 4): [lambda: emit_outproj(0, ots=H1)],
                (4, 5): [lambda: emit_proj("wq", qTr, 1, 1)],
                (4, 6): [lambda: emit_outproj(0, ots=H2)],
                (5, 4): [lambda: emit_outproj(3, kts=(0,), dst_t=io["out_t2"], ots=H1)],
                (5, 5): [lambda: emit_proj("wq", qTr, 1, 2)],
                (5, 6): [lambda: emit_outproj(3, kts=(0,), dst_t=io["out_t2"], ots=H2)],
                (6, 4): [lambda: emit_outproj(1, ots=H1)],
                (6, 5): [lambda: emit_proj("wq", qTr, 1, 3)],
                (6, 6): [lambda: emit_outproj(1, ots=H2)],
                (7, 4): [lambda: emit_outproj(2, ots=H1)],
                (7, 6): [lambda: emit_outproj(2, ots=H2)],
            }

            def norm_one(mt, qsl, at, hl):
                rcr = rcpp.tile([1, 2, 512], F32, tag="rc")
                nc.vector.tensor_copy(out=rcr[:, 0], in_=at[DEPTH : DEPTH + 1, :])
                nc.vector.reciprocal_approx_fast(out=rcr[:, 1], in_=rcr[:, 0])
                bc = bcp.tile([DEPTH, 512], F32, tag="bc")
                nc.gpsimd.partition_broadcast(bc, rcr[:, 1])
                po = hl * DEPTH
                nc.vector.tensor_mul(
                    out=attn_sb[mt][po : po + DEPTH, qsl],
                    in0=at[0:DEPTH, :],
                    in1=bc,
                )

            def make_normalize(mt, qsl, ats):
                """Deferred: emitted one pass later so the recip chain and
                the at-psum reuse never head-block the tensor queue."""
                def normalize():
                    for hl in range(2):
                        norm_one(mt, qsl, ats[hl], hl)
                return normalize

            # mt1's K/Q are produced during the first passes -> mt0 passes
            # first, with (0,1) fourth so K(mt1) has ~3 passes of slack.
            passes = [(0, 0), (1, 0), (2, 0), (0, 1), (3, 0), (1, 1), (2, 1), (3, 1)]
            norm_prev = None
            carry = []          # previous pass's trailing AV groups
            for pi, (qc, mt) in enumerate(passes):
                qsl = bass.ts(qc, 512)
                # at-psum tiles allocated lazily at the first AV emission so
                # the pool generation starts after the previous pass's
                # carried AV writers are in the program.
                ats_box = []

                def get_ats(ats_box=ats_box):
                    if not ats_box:
                        ats_box.extend(
                            atp.tile(
                                [DEPTH + 1, 512], F32, tag=f"at{hl}", name=f"at{hl}"
                            )
                            for hl in range(2)
                        )
                    return ats_box

                def emit_av(ktp, wt, get_ats=get_ats, mt=mt, hls=(0, 1)):
                    ats = get_ats()
                    for j in range(2):
                        kt = 2 * ktp + j
                        for hl in hls:
                            nc.tensor.matmul(
                                ats[hl],
                                lhsT=v_sb[:, kt, 2 * mt + hl, :],
                                rhs=wt[:, j, hl, :],
                                start=(kt == 0),
                                stop=(kt == TT - 1),
                            )

                pend = []
                for ktp in range(KTP):
                    wt = wtp.tile([P, 2, 2, 512], BF16, tag="wt")
                    for j in range(2):
                        kt = 2 * ktp + j
                        lg = lgp.tile([P, 2, 512], F32, tag="lg")
                        # head pair in disjoint PE row-groups -> concurrent
                        for hl in range(2):
                            po = hl * DEPTH
                            nc.tensor.matmul(
                                lg[:, hl],
                                lhsT=kTr[mt][po : po + DEPTH, kt * P : (kt + 1) * P],
                                rhs=qTr[mt][po : po + DEPTH, qsl],
                                start=True,
                                stop=True,
                            )
                        nc.scalar.activation(
                            out=wt[:, j], in_=lg, func=EXP, scale=0.125
                        )
                    if ktp == 2:
                        # previous pass's trailing AVs + its normalize, far
                        # enough in that ~4 exps cover the latency.
                        for fn in carry:
                            fn()
                        carry = []
                        if norm_prev is not None:
                            norm_prev()
                            norm_prev = None
                    for fn in extra.get((pi, ktp), ()):
                        fn()
                    # AV runs 3 ktps late so at-psum reuse and the normalize
                    # chain hide behind queued exps.
                    pend.append((ktp, wt))
                    if len(pend) > 3:
                        emit_av(*pend.pop(0))
                carry = [
                    (lambda k=k, w=w, f=emit_av: f(k, w)) for k, w in pend
                ]
                norm_prev = make_normalize(mt, qsl, get_ats())
            # tail: flush the last pass hl-major so each half's normalize
            # overlaps the other half's AV matmuls.
            qsl = bass.ts(passes[-1][0], 512)
            for hl in range(2):
                for k, w in pend:
                    emit_av(k, w, hls=(hl,))
                norm_one(passes[-1][1], qsl, get_ats()[hl], hl)
            emit_outproj(3, kts=(1,))


def _build():
    nc = bacc.Bacc(
        "TRN2", target_bir_lowering=False, debug=False, num_devices=N_CORES
    )
    io = {
        "qt": nc.dram_tensor("qt", (P, KT, S), BF16, kind="ExternalInput").ap(),
        "wq": nc.dram_tensor("wq", (P, KT, HD), BF16, kind="ExternalInput").ap(),
        "wk": nc.dram_tensor("wk", (P, KT, HD), BF16, kind="ExternalInput").ap(),
        "wv": nc.dram_tensor("wv", (P, KT, HD), BF16, kind="ExternalInput").ap(),
        "wo": nc.dram_tensor("wo", (P, 2, D), BF16, kind="ExternalInput").ap(),
        "bq": nc.dram_tensor("bq", (P, 2), F32, kind="ExternalInput").ap(),
        "bk": nc.dram_tensor("bk", (P, 2), F32, kind="ExternalInput").ap(),
        "cost": nc.dram_tensor("cost", (P, 2, S), BF16, kind="ExternalInput").ap(),
        "sint": nc.dram_tensor("sint", (P, 2, S), BF16, kind="ExternalInput").ap(),
        "rotm": nc.dram_tensor("rotm", (P, P), BF16, kind="ExternalInput").ap(),
        "bout": nc.dram_tensor("bout", (P, KT), F32, kind="ExternalInput").ap(),
        "out_t": nc.dram_tensor("out_t", (D, S), BF16, kind="ExternalOutput").ap(),
        "out_t2": nc.dram_tensor("out_t2", (D, 512), BF16, kind="ExternalOutput").ap(),
    }
    with tile.TileContext(nc) as tc:
        _mha_tile(tc, io)
    nc.compile()
    return nc


def _get_built():
    global _BUILT
    if _BUILT is None:
        _BUILT = _build()
    return _BUILT


def _trig():
    inv_freq = 1.0 / (10000.0 ** (np.arange(0, DEPTH, 2, dtype=np.float64) / DEPTH))
    t = np.arange(S, dtype=np.float64)
    freqs = np.outer(t, inv_freq)             # [S, 32]
    emb = np.concatenate([freqs, freqs], 1)   # [S, 64]
    return (
        np.cos(emb).T.astype(np.float32),     # [64, S]
        np.sin(emb).T.astype(np.float32),
    )


def _pm(a, groups):
    """[groups*P, X...] row-major -> partition-major [P, groups, X...]."""
    return np.ascontiguousarray(
        a.reshape(groups, P, *a.shape[1:]).transpose(1, 0, *range(2, a.ndim + 1))
    )


def _host_inputs(inputs):
    query = np.asarray(inputs["query"], np.float32)
    Wq = np.asarray(inputs["Wq"], np.float32)
    Wk = np.asarray(inputs["Wk"], np.float32)
    Wv = np.asarray(inputs["Wv"], np.float32)
    Wo = np.asarray(inputs["Wo"], np.float32)
    bq = np.asarray(inputs["bq"], np.float32)
    bk = np.asarray(inputs["bk"], np.float32)
    bv = np.asarray(inputs["bv"], np.float32)
    bo = np.asarray(inputs["bo"], np.float32)
    gamma = np.asarray(inputs["gamma"], np.float32)
    # mask is all-True by construction (fill: ones); softmax masking is a no-op.

    qt_b = [
        _pm(np.ascontiguousarray(query[b].T), KT).astype(BF16_NP) for b in range(B)
    ]
    WqT, WkT, WvT, WoT = Wq.T, Wk.T, Wv.T, Wo.T

    cosT, sinT = _trig()
    sinS = sinT.copy()
    sinS[: DEPTH // 2] *= -1.0  # sign for the -x2 half of rotate_half
    cost_full = _pm(np.ascontiguousarray(np.tile(cosT, (HPC, 1))), 2).astype(BF16_NP)
    sint_full = _pm(np.ascontiguousarray(np.tile(sinS, (HPC, 1))), 2).astype(BF16_NP)

    rotm = np.zeros((P, P), np.float32)
    m = np.arange(P)
    rotm[(m // DEPTH) * DEPTH + (m % DEPTH + DEPTH // 2) % DEPTH, m] = 1.0
    rotm = rotm.astype(BF16_NP)

    in_maps = []
    for c in range(N_CORES):
        b, hg = divmod(c, HPC)
        sl = slice(hg * HD, (hg + 1) * HD)
        bout_c = gamma * (bv[sl] @ WoT[sl, :])
        if hg == 0:
            bout_c = bout_c + gamma * bo
        in_maps.append(
            {
                "qt": qt_b[b],
                "wq": _pm(np.ascontiguousarray(WqT[:, sl]), KT).astype(BF16_NP),
                "wk": _pm(np.ascontiguousarray(WkT[:, sl]), KT).astype(BF16_NP),
                "wv": _pm(np.ascontiguousarray(WvT[:, sl]), KT).astype(BF16_NP),
                "wo": _pm(
                    np.ascontiguousarray(WoT[sl, :] * gamma[None, :]), 2
                ).astype(BF16_NP),
                "bq": _pm(np.ascontiguousarray(bq[sl]), 2),
                "bk": _pm(np.ascontiguousarray(bk[sl]), 2),
                "cost": cost_full,
                "sint": sint_full,
                "rotm": rotm,
                "bout": _pm(np.ascontiguousarray(bout_c.astype(np.float32)), KT),
            }
        )
    return in_maps


def _gather(results):
    out = np.empty((B, S, D), np.float32)
    for b in range(B):
        acc = None
        for hg in range(HPC):
            r = results[b * HPC + hg]
            part = r["out_t"].astype(np.float32)
            part[:, 3 * 512 :] += r["out_t2"].astype(np.float32)
            acc = part if acc is None else acc + part
        out[b] = acc.T
    return out


def kernel(**inputs) -> np.ndarray:
    nc = _get_built()
    in_maps = _host_inputs(inputs)
    res = run_bass_kernel_spmd(nc, in_maps, core_ids=list(range(N_CORES)))
    return _gather(res.results)


# exposed for test.py (profiling path)
def run_with_results(inputs, **kw):
    nc = _get_built()
    in_maps = _host_inputs(inputs)
    res = run_bass_kernel_spmd(nc, in_maps, core_ids=list(range(N_CORES)), **kw)
    return _gather(res.results), res


# revision 38
# speedup vs baseline: 1.0012x; 1.0012x over previous
"""Multi-head attention (RoPE, dense mask) Trainium2 Bass kernel.

Problem: B=2, S=2048, D=1024, H=16 heads of depth 64.
  q/k/v = query @ W{q,k,v}.T + b   (RoPE on q,k)   -> softmax(q k^T / 8) v
  out = gamma * (attn @ Wo.T + bo)

Sharding over 8 cores: batch (2) x head-groups (4 heads = 256 dims each).
Each core computes its batch's attention for its 4 heads plus the partial
row-parallel out-projection; host sums the 4 partials per batch.

The kernel is softmax-exp bound: 16.8M exps/core on the scalar (ACT)
engine at 1 elem/lane/cycle (1.2 GHz) is ~135us. The design keeps ACT
busy on exp and hides everything else under it:
  - all matmuls bf16 (fp8 anywhere in the q/k/wt/v path costs ~2.5e-2
    rel err: attention output is a weighted average, so quantization
    noise does NOT average down relative to the signal).
  - logits (contraction=64) issue head-pairs back-to-back into disjoint
    PE row-groups (partitions 0-63 / 64-127): the two matmuls run
    concurrently in the array (measured dt_start ~4ns).
  - projections are pipelined INTO the attention passes: pass (qc,mt)
    consumes K-chunks incrementally per k-tile-pair, so K(mt1)/Q/V
    projections are emitted as 'extra' work inside earlier passes.
    Evacuations/bias-adds run on DVE/GPSIMD, never ACT.
  - AV matmuls are emitted 2 k-tile-pairs late so the pass-boundary
    at-psum reuse (next AV start waits previous normalize) hides
    behind ~4us of queued exps.
  - all dram inputs are host-swizzled partition-major so every DMA has
    >=2KB contiguous per-partition lines (~3x DMA throughput).
  - out-projection (bf16 partials) + bf16 output DMA per q-chunk.
"""

import numpy as np
import ml_dtypes

import concourse.bass as bass
import concourse.tile as tile
from concourse import bacc, mybir
from concourse.bass_utils import run_bass_kernel_spmd

B, S, D, H, DEPTH = 2, 2048, 1024, 16, 64
N_CORES = 8
HPC = 4            # heads per core
HD = HPC * DEPTH   # 256 head-dims per core
P = 128
KT = D // P        # 8 contraction tiles for the projections
NCH = S // 512     # 4 chunks of 512
TT = S // P        # 16 token/key tiles
KTP = TT // 2      # 8 k-tile pairs per pass
F32 = mybir.dt.float32
BF16 = mybir.dt.bfloat16
EXP = mybir.ActivationFunctionType.Exp
BF16_NP = ml_dtypes.bfloat16

_BUILT = None


def _mha_tile(tc, io):
    nc = tc.nc
    qt, wq, wk, wv, wo = io["qt"], io["wq"], io["wk"], io["wv"], io["wo"]
    bq, bk, cost, sint = io["bq"], io["bk"], io["cost"], io["sint"]
    rotm, bout, out_t = io["rotm"], io["bout"], io["out_t"]

    with tc.tile_pool(name="persist", bufs=1) as persist:
        qTr = [persist.tile([P, S], BF16, tag=f"qTr{m}", name=f"qTr{m}") for m in range(2)]
        kTr = [persist.tile([P, S], BF16, tag=f"kTr{m}", name=f"kTr{m}") for m in range(2)]
        # token-major V with an all-ones 65th column per head (denominator).
        # One tile per 128-token tile: dependency tracking is tile-granular,
        # so a shared tile would stall each AV on unrelated later V writes.
        v_tiles = [
            persist.tile([P, HPC, DEPTH + 1], BF16, tag=f"v{tt}", name=f"v{tt}")
            for tt in range(TT)
        ]
        for tt in range(TT):
            nc.vector.memset(v_tiles[tt][:, :, DEPTH : DEPTH + 1], 1.0)
        attn_sb = [persist.tile([P, S], BF16, tag=f"attn{m}", name=f"attn{m}") for m in range(2)]

        # ---- input DMA, ordered for earliest K(mt0) start ----
        rotm_sb = persist.tile([P, P], BF16, tag="rotm")
        nc.sync.dma_start(out=rotm_sb, in_=rotm)
        w_sbs = {}
        for name, w, b in (("wk", wk, bk), ("wq", wq, bq), ("wv", wv, None)):
            w_sbs[name] = persist.tile([P, KT, HD], BF16, tag=name, name=name)
            if b is not None:
                w_sbs[name + "_b"] = persist.tile(
                    [P, 2], F32, tag=name + "_b", name=name + "_b"
                )
        # separate tiles per token-half / mt-half: avoids false waits on the
        # later DMA pieces (tile-granular dependency tracking)
        cos_t = [persist.tile([P, S], BF16, tag=f"cos{m}", name=f"cos{m}") for m in range(2)]
        sin_t = [persist.tile([P, S], BF16, tag=f"sin{m}", name=f"sin{m}") for m in range(2)]
        qt_h = [
            persist.tile([P, KT, 1024], BF16, tag=f"qth{h}", name=f"qth{h}")
            for h in range(2)
        ]
        wo_sb = persist.tile([P, 2, D], BF16, tag="wo")
        bout_sb = persist.tile([P, KT], F32, tag="bout")

        nc.sync.dma_start(out=w_sbs["wk"], in_=wk)
        nc.sync.dma_start(out=w_sbs["wk_b"], in_=bk)
        for kt in range(KT):
            nc.sync.dma_start(out=qt_h[0][:, kt, :], in_=qt[:, kt, 0:1024])
        nc.sync.dma_start(out=cos_t[0], in_=cost[:, 0])
        nc.sync.dma_start(out=sin_t[0], in_=sint[:, 0])
        nc.sync.dma_start(out=w_sbs["wq"], in_=wq)
        nc.sync.dma_start(out=w_sbs["wq_b"], in_=bq)
        nc.sync.dma_start(out=w_sbs["wv"], in_=wv)
        for kt in range(KT):
            nc.sync.dma_start(out=qt_h[1][:, kt, :], in_=qt[:, kt, 1024:2048])
        nc.sync.dma_start(out=cos_t[1], in_=cost[:, 1])
        nc.sync.dma_start(out=sin_t[1], in_=sint[:, 1])
        nc.sync.dma_start(out=wo_sb, in_=wo)
        nc.sync.dma_start(out=bout_sb, in_=bout)

        with (
            tc.tile_pool(name="wt", bufs=8) as wtp,
            tc.tile_pool(name="bc", bufs=3) as bcp,
            tc.tile_pool(name="rcp", bufs=3) as rcpp,
            tc.tile_pool(name="oc", bufs=3) as ocp,
            tc.tile_pool(name="lg_ps", bufs=2, space="PSUM") as lgp,
            tc.tile_pool(name="at_ps", bufs=1, space="PSUM") as atp,
            tc.tile_pool(name="ms_ps", bufs=2, space="PSUM") as msp,
        ):
            # PE warm-up: HAM clock-gate ramps to 8/8 while input DMAs land.
            warm = msp.tile([P, 512], F32, tag="ms", name="warm")
            for _ in range(24):
                nc.tensor.matmul(
                    warm[:, 0:P], lhsT=rotm_sb, rhs=rotm_sb,
                    start=True, stop=True, skip_group_check=True,
                )

            def emit_proj(wname, dstpair, mt, ch):
                """Q/K projection + RoPE for one (mt half, 512-token chunk)."""
                w_sb, b_sb = w_sbs[wname], w_sbs[wname + "_b"]
                sl = bass.ts(ch, 512)
                hsl = bass.ts(ch % 2, 512)
                qth = qt_h[ch // 2]
                dst = dstpair[mt]
                ps = msp.tile([P, 512], F32, tag="ms")
                for kt in range(KT):
                    nc.tensor.matmul(
                        ps,
                        lhsT=w_sb[:, kt, mt * P : (mt + 1) * P],
                        rhs=qth[:, kt, hsl],
                        start=(kt == 0),
                        stop=(kt == KT - 1),
                    )
                # evacuate + bias on DVE (keep ACT free for exp)
                nc.vector.tensor_scalar_add(
                    out=dst[:, sl], in0=ps, scalar1=b_sb[:, mt : mt + 1]
                )
                rps = msp.tile([P, 512], F32, tag="ms")
                nc.tensor.matmul(
                    rps, lhsT=rotm_sb, rhs=dst[:, sl], start=True, stop=True
                )
                # x' = x*cos + rot(x)*sin_signed  (all DVE: gpsimd must run a
                # single ucode -- partition_broadcast -- or each program
                # switch costs ~7us of Q7 reload).  cos-mul goes to a temp so
                # it runs during the rot matmul instead of serializing.
                ct = ocp.tile([P, 512], BF16, tag="ct")
                nc.vector.tensor_mul(out=ct, in0=dst[:, sl], in1=cos_t[mt][:, sl])
                nc.vector.tensor_mul(out=rps, in0=rps, in1=sin_t[mt][:, sl])
                nc.vector.tensor_add(out=dst[:, sl], in0=ct, in1=rps)

            def emit_v(tt):
                """V projection for one 128-token tile -> bf16 token-major."""
                qth = qt_h[tt // 8]
                tsl = bass.ts(tt % 8, P)
                ps = msp.tile([P, 512], F32, tag="ms")
                for kt in range(KT):
                    nc.tensor.matmul(
                        ps[:, 0:HD],
                        lhsT=qth[:, kt, tsl],
                        rhs=w_sbs["wv"][:, kt, :],
                        start=(kt == 0),
                        stop=(kt == KT - 1),
                    )
                nc.vector.tensor_copy(
                    out=v_tiles[tt][:, :, 0:DEPTH],
                    in_=ps[:, 0:HD].rearrange("p (h d) -> p h d", h=HPC),
                )

            def emit_outproj(qc, kts=(0, 1), dst_t=None, ots=range(KT), tail=False):
                """Out-projection partial over attn mt-halves `kts`.  qc3 is
                split: the mt0 half runs a pass early into out_t2 (host adds
                it), shortening the tail to the mt1 half only.  In the tail,
                evacuations alternate ACT/DVE (ACT is idle by then)."""
                qsl = bass.ts(qc, 512)
                dst_t = out_t if dst_t is None else dst_t
                for ot in ots:
                    ps = msp.tile([P, 512], F32, tag="ms")
                    for kt in kts:
                        nc.tensor.matmul(
                            ps,
                            lhsT=wo_sb[:, kt, ot * P : (ot + 1) * P],
                            rhs=attn_sb[kt][:, qsl],
                            start=(kt == kts[0]),
                            stop=(kt == kts[-1]),
                        )
                    ob = ocp.tile([P, 512], BF16, tag="ob")
                    if tail and ot % 2 == 0:
                        nc.scalar.add(
                            out=ob, in_=ps, add=bout_sb[:, ot : ot + 1]
                        )
                    elif kts[-1] == 1:
                        nc.vector.tensor_scalar_add(
                            out=ob, in0=ps, scalar1=bout_sb[:, ot : ot + 1]
                        )
                    else:
                        nc.vector.tensor_copy(out=ob, in_=ps)
                    nc.sync.dma_start(
                        out=dst_t[ot * P : (ot + 1) * P, qc * 512 : (qc + 1) * 512]
                        if dst_t is out_t
                        else dst_t[ot * P : (ot + 1) * P, :],
                        in_=ob,
                    )

            # ---- lead-in: K(mt0, ch0), Q(mt0, ch0) only ----
            # attention passes consume K-chunks and V-tiles incrementally;
            # everything else is emitted as 'extra' work inside the passes.
            emit_proj("wk", kTr, 0, 0)
            emit_proj("wq", qTr, 0, 0)

            # extra work emitted inside attention passes (after each ktp's
            # logits/exps so the ACT queue is fed first): (pass, ktp) -> fns
            H1, H2 = range(0, 4), range(4, KT)
            extra = {
                (0, 0): [lambda: emit_v(0), lambda: emit_v(1)],
                (0, 1): [lambda: emit_proj("wk", kTr, 0, 1),
                         lambda: emit_v(2), lambda: emit_v(3)],
                (0, 2): [lambda: emit_v(4), lambda: emit_v(5)],
                (0, 3): [lambda: emit_proj("wk", kTr, 0, 2)],
                (0, 4): [lambda: emit_v(6), lambda: emit_v(7)],
                (0, 5): [lambda: emit_proj("wk", kTr, 0, 3),
                         lambda: emit_v(8), lambda: emit_v(9)],
                (0, 6): [lambda: emit_v(10), lambda: emit_v(11)],
                (0, 7): [lambda: emit_v(12), lambda: emit_v(13),
                         lambda: emit_proj("wq", qTr, 0, 1)],
                (1, 0): [lambda: emit_v(14), lambda: emit_v(15)],
                (1, 5): [lambda: emit_proj("wk", kTr, 1, 0)],
                (1, 6): [lambda: emit_proj("wk", kTr, 1, 1)],
                (1, 7): [lambda: emit_proj("wq", qTr, 0, 2)],
                (2, 0): [lambda: emit_proj("wk", kTr, 1, 2)],
                (2, 1): [lambda: emit_proj("wk", kTr, 1, 3)],
                (2, 5): [lambda: emit_proj("wq", qTr, 1, 0)],
                (3, 5): [lambda: emit_proj("wq", qTr, 0, 3)],
                (4, 4): [lambda: emit_outproj(0, ots=H1)],
                (4, 5): [lambda: emit_proj("wq", qTr, 1, 1)],
                (4, 6): [lambda: emit_outproj(0, ots=H2)],
                (5, 4): [lambda: emit_outproj(3, kts=(0,), dst_t=io["out_t2"], ots=H1)],
                (5, 5): [lambda: emit_proj("wq", qTr, 1, 2)],
                (5, 6): [lambda: emit_outproj(3, kts=(0,), dst_t=io["out_t2"], ots=H2)],
                (6, 4): [lambda: emit_outproj(1, ots=H1)],
                (6, 5): [lambda: emit_proj("wq", qTr, 1, 3)],
                (6, 6): [lambda: emit_outproj(1, ots=H2)],
                (7, 4): [lambda: emit_outproj(2, ots=H1)],
                (7, 6): [lambda: emit_outproj(2, ots=H2)],
            }

            def norm_one(mt, qsl, at, hl):
                rcr = rcpp.tile([1, 2, 512], F32, tag="rc")
                nc.vector.tensor_copy(out=rcr[:, 0], in_=at[DEPTH : DEPTH + 1, :])
                nc.vector.reciprocal_approx_fast(out=rcr[:, 1], in_=rcr[:, 0])
                bc = bcp.tile([DEPTH, 512], F32, tag="bc")
                nc.gpsimd.partition_broadcast(bc, rcr[:, 1])
                po = hl * DEPTH
                nc.vector.tensor_mul(
                    out=attn_sb[mt][po : po + DEPTH, qsl],
                    in0=at[0:DEPTH, :],
                    in1=bc,
                )

            def make_normalize(mt, qsl, ats):
                """Deferred: emitted one pass later so the recip chain and
                the at-psum reuse never head-block the tensor queue."""
                def normalize():
                    for hl in range(2):
                        norm_one(mt, qsl, ats[hl], hl)
                return normalize

            # mt1's K/Q are produced during the first passes -> mt0 passes
            # first, with (0,1) fourth so K(mt1) has ~3 passes of slack.
            passes = [(0, 0), (1, 0), (2, 0), (0, 1), (3, 0), (1, 1), (2, 1), (3, 1)]
            norm_prev = None
            carry = []          # previous pass's trailing AV groups
            for pi, (qc, mt) in enumerate(passes):
                qsl = bass.ts(qc, 512)
                # at-psum tiles allocated lazily at the first AV emission so
                # the pool generation starts after the previous pass's
                # carried AV writers are in the program.
                ats_box = []

                def get_ats(ats_box=ats_box):
                    if not ats_box:
                        ats_box.extend(
                            atp.tile(
                                [DEPTH + 1, 512], F32, tag=f"at{hl}", name=f"at{hl}"
                            )
                            for hl in range(2)
                        )
                    return ats_box

                def emit_av(ktp, wt, get_ats=get_ats, mt=mt, hls=(0, 1)):
                    ats = get_ats()
                    for j in range(2):
                        kt = 2 * ktp + j
                        for hl in hls:
                            nc.tensor.matmul(
                                ats[hl],
                                lhsT=v_tiles[kt][:, 2 * mt + hl, :],
                                rhs=wt[:, j, hl, :],
                                start=(kt == 0),
                                stop=(kt == TT - 1),
                            )

                pend = []
                for ktp in range(KTP):
                    wt = wtp.tile([P, 2, 2, 512], BF16, tag="wt")
                    for j in range(2):
                        kt = 2 * ktp + j
                        lg = lgp.tile([P, 2, 512], F32, tag="lg")
                        # head pair in disjoint PE row-groups -> concurrent
                        for hl in range(2):
                            po = hl * DEPTH
                            nc.tensor.matmul(
                                lg[:, hl],
                                lhsT=kTr[mt][po : po + DEPTH, kt * P : (kt + 1) * P],
                                rhs=qTr[mt][po : po + DEPTH, qsl],
                                start=True,
                                stop=True,
                            )
                        nc.scalar.activation(
                            out=wt[:, j], in_=lg, func=EXP, scale=0.125
                        )
                    if ktp == 2:
                        # previous pass's trailing AVs + its normalize, far
                        # enough in that ~4 exps cover the latency.
                        for fn in carry:
                            fn()
                        carry = []
                        if norm_prev is not None:
                            norm_prev()
                            norm_prev = None
                    for fn in extra.get((pi, ktp), ()):
                        fn()
                    # AV runs 3 ktps late so at-psum reuse and the normalize
                    # chain hide behind queued exps.
                    pend.append((ktp, wt))
                    if len(pend) > 3:
                        emit_av(*pend.pop(0))
                carry = [
                    (lambda k=k, w=w, f=emit_av: f(k, w)) for k, w in pend
                ]
                norm_prev = make_normalize(mt, qsl, get_ats())
            # tail: flush the last pass hl-major so each half's normalize
            # overlaps the other half's AV matmuls.
            qsl = bass.ts(passes[-1][0], 512)
            for hl in range(2):
                for k, w in pend:
                    emit_av(k, w, hls=(hl,))
                norm_one(passes[-1][1], qsl, get_ats()[hl], hl)
            emit_outproj(3, kts=(1,), tail=True)


def _build():
    nc = bacc.Bacc(
        "TRN2", target_bir_lowering=False, debug=False, num_devices=N_CORES
    )
    io = {
        "qt": nc.dram_tensor("qt", (P, KT, S), BF16, kind="ExternalInput").ap(),
        "wq": nc.dram_tensor("wq", (P, KT, HD), BF16, kind="ExternalInput").ap(),
        "wk": nc.dram_tensor("wk", (P, KT, HD), BF16, kind="ExternalInput").ap(),
        "wv": nc.dram_tensor("wv", (P, KT, HD), BF16, kind="ExternalInput").ap(),
        "wo": nc.dram_tensor("wo", (P, 2, D), BF16, kind="ExternalInput").ap(),
        "bq": nc.dram_tensor("bq", (P, 2), F32, kind="ExternalInput").ap(),
        "bk": nc.dram_tensor("bk", (P, 2), F32, kind="ExternalInput").ap(),
        "cost": nc.dram_tensor("cost", (P, 2, S), BF16, kind="ExternalInput").ap(),
        "sint": nc.dram_tensor("sint", (P, 2, S), BF16, kind="ExternalInput").ap(),
        "rotm": nc.dram_tensor("rotm", (P, P), BF16, kind="ExternalInput").ap(),
        "bout": nc.dram_tensor("bout", (P, KT), F32, kind="ExternalInput").ap(),
        "out_t": nc.dram_tensor("out_t", (D, S), BF16, kind="ExternalOutput").ap(),
        "out_t2": nc.dram_tensor("out_t2", (D, 512), BF16, kind="ExternalOutput").ap(),
    }
    with tile.TileContext(nc) as tc:
        _mha_tile(tc, io)
    nc.compile()
    return nc


def _get_built():
    global _BUILT
    if _BUILT is None:
        _BUILT = _build()
    return _BUILT


def _trig():
    inv_freq = 1.0 / (10000.0 ** (np.arange(0, DEPTH, 2, dtype=np.float64) / DEPTH))
    t = np.arange(S, dtype=np.float64)
    freqs = np.outer(t, inv_freq)             # [S, 32]
    emb = np.concatenate([freqs, freqs], 1)   # [S, 64]
    return (
        np.cos(emb).T.astype(np.float32),     # [64, S]
        np.sin(emb).T.astype(np.float32),
    )


def _pm(a, groups):
    """[groups*P, X...] row-major -> partition-major [P, groups, X...]."""
    return np.ascontiguousarray(
        a.reshape(groups, P, *a.shape[1:]).transpose(1, 0, *range(2, a.ndim + 1))
    )


def _host_inputs(inputs):
    query = np.asarray(inputs["query"], np.float32)
    Wq = np.asarray(inputs["Wq"], np.float32)
    Wk = np.asarray(inputs["Wk"], np.float32)
    Wv = np.asarray(inputs["Wv"], np.float32)
    Wo = np.asarray(inputs["Wo"], np.float32)
    bq = np.asarray(inputs["bq"], np.float32)
    bk = np.asarray(inputs["bk"], np.float32)
    bv = np.asarray(inputs["bv"], np.float32)
    bo = np.asarray(inputs["bo"], np.float32)
    gamma = np.asarray(inputs["gamma"], np.float32)
    # mask is all-True by construction (fill: ones); softmax masking is a no-op.

    qt_b = [
        _pm(np.ascontiguousarray(query[b].T), KT).astype(BF16_NP) for b in range(B)
    ]
    WqT, WkT, WvT, WoT = Wq.T, Wk.T, Wv.T, Wo.T

    cosT, sinT = _trig()
    sinS = sinT.copy()
    sinS[: DEPTH // 2] *= -1.0  # sign for the -x2 half of rotate_half
    cost_full = _pm(np.ascontiguousarray(np.tile(cosT, (HPC, 1))), 2).astype(BF16_NP)
    sint_full = _pm(np.ascontiguousarray(np.tile(sinS, (HPC, 1))), 2).astype(BF16_NP)

    rotm = np.zeros((P, P), np.float32)
    m = np.arange(P)
    rotm[(m // DEPTH) * DEPTH + (m % DEPTH + DEPTH // 2) % DEPTH, m] = 1.0
    rotm = rotm.astype(BF16_NP)

    in_maps = []
    for c in range(N_CORES):
        b, hg = divmod(c, HPC)
        sl = slice(hg * HD, (hg + 1) * HD)
        bout_c = gamma * (bv[sl] @ WoT[sl, :])
        if hg == 0:
            bout_c = bout_c + gamma * bo
        in_maps.append(
            {
                "qt": qt_b[b],
                "wq": _pm(np.ascontiguousarray(WqT[:, sl]), KT).astype(BF16_NP),
                "wk": _pm(np.ascontiguousarray(WkT[:, sl]), KT).astype(BF16_NP),
                "wv": _pm(np.ascontiguousarray(WvT[:, sl]), KT).astype(BF16_NP),
                "wo": _pm(
                    np.ascontiguousarray(WoT[sl, :] * gamma[None, :]), 2
                ).astype(BF16_NP),
                "bq": _pm(np.ascontiguousarray(bq[sl]), 2),
                "bk": _pm(np.ascontiguousarray(bk[sl]), 2),
                "cost": cost_full,
                "sint": sint_full,
                "rotm": rotm,
                "bout": _pm(np.ascontiguousarray(bout_c.astype(np.float32)), KT),
            }
        )
    return in_maps


def _gather(results):
    out = np.empty((B, S, D), np.float32)
    for b in range(B):
        acc = None
        for hg in range(HPC):
            r = results[b * HPC + hg]
            part = r["out_t"].astype(np.float32)
            part[:, 3 * 512 :] += r["out_t2"].astype(np.float32)
            acc = part if acc is None else acc + part
        out[b] = acc.T
    return out


def kernel(**inputs) -> np.ndarray:
    nc = _get_built()
    in_maps = _host_inputs(inputs)
    res = run_bass_kernel_spmd(nc, in_maps, core_ids=list(range(N_CORES)))
    return _gather(res.results)


# exposed for test.py (profiling path)
def run_with_results(inputs, **kw):
    nc = _get_built()
    in_maps = _host_inputs(inputs)
    res = run_bass_kernel_spmd(nc, in_maps, core_ids=list(range(N_CORES)), **kw)
    return _gather(res.results), res
